# revision 1
# baseline (speedup 1.0000x reference)
"""Trainium2 Bass kernel for the CMA momentum-memory update (nn_CMA_52956946760162).

Strategy: shard the C=4096 classes across 8 cores (512 classes/core). On host,
gather each core's feature rows by label range, sort by (label, cam) segment id,
and pad each 128-segment chunk to a fixed row capacity. The device computes
per-(label,cam) segment sums as one-hot matmuls on the tensor engine (the
one-hot entries are pre-scaled by the momentum/count coefficient b so PSUM
directly holds b*csum), then blends with the memory banks via a single fused
DVE op per tile: out = a*mem + psum. Per-label sums reuse the same rows with a
class-level one-hot accumulated in PSUM across the 6 segment chunks that make
up each 128-class chunk. No collectives: cores are fully independent.
"""

import numpy as np

C, K, D, N = 4096, 6, 2048, 16384
SIGMA = 0.2
M = 8                 # cores
CPC = C // M          # classes per core = 512
SEGS = CPC * K        # (label,cam) segments per core = 3072
NCH = SEGS // 128     # segment chunks per core = 24
NCC = CPC // 128      # class chunks per core = 4 (each spans 6 segment chunks)
F32 = np.float32

_BUILD_CACHE = {}


def _prep_core_modality(core, feats, labels, cams, valid, B):
    """Host-side prep for one (core, modality). Returns dict of device arrays."""
    c0 = core * CPC
    mask = (labels >= c0) & (labels < c0 + CPC)
    rows = np.nonzero(mask)[0]
    seg = (labels[rows] - c0) * K + cams[rows]
    order = np.argsort(seg, kind="stable")
    rows, seg = rows[order], seg[order]

    ch = seg // 128
    cnt_per_chunk = np.bincount(ch, minlength=NCH)
    assert cnt_per_chunk.max() <= B * 128

    ccnt = np.bincount(seg, minlength=SEGS).astype(F32)
    gcnt = np.bincount(labels[rows] - c0, minlength=CPC).astype(F32)
    v = np.asarray(valid[c0:c0 + CPC]).reshape(SEGS)
    cpres = ccnt > 0
    a_c = np.where(cpres, np.where(v, 1.0 - SIGMA, 0.0), 1.0).astype(F32)
    b_c = np.where(cpres, np.where(v, SIGMA, 1.0) / np.maximum(ccnt, 1.0), 0.0).astype(F32)
    gpres = gcnt > 0
    a_g = np.where(gpres, 1.0 - SIGMA, 1.0).astype(F32)
    b_g = np.where(gpres, SIGMA / np.maximum(gcnt, 1.0), 0.0).astype(F32)

    Fpad = np.zeros((NCH, B * 128, D), F32)
    ohc = np.zeros((NCH, B * 128, 128), F32)
    ohg = np.zeros((NCH, B * 128, 128), F32)
    starts = np.concatenate([[0], np.cumsum(cnt_per_chunk)])
    for j in range(NCH):
        r = rows[starts[j]:starts[j + 1]]
        s = seg[starts[j]:starts[j + 1]]
        n = len(r)
        if n == 0:
            continue
        Fpad[j, :n] = feats[r]
        k = np.arange(n)
        ohc[j, k, s - 128 * j] = b_c[s]
        cloc = s // K - 128 * (j // 6)          # class col within class chunk j//6
        ohg[j, k, cloc] = b_g[s // K]

    return dict(
        fpad=Fpad.reshape(NCH * B * 128, D),
        ohc=ohc,
        ohg=ohg,
        acs=np.ascontiguousarray(a_c.reshape(NCH, 128).T),   # [128, NCH]
        ags=np.ascontiguousarray(a_g.reshape(NCC, 128).T),   # [128, NCC]
    )


def _build_program(B):
    """Build + compile the SPMD Bass program for row capacity B*128 per chunk."""
    import concourse.mybir as mybir
    import concourse.tile as tile
    from concourse import bacc

    f32 = mybir.dt.float32
    nc = bacc.Bacc("TRN2", target_bir_lowering=False, debug=False)

    ins = {}
    for m in range(2):
        ins[f"fpad{m}"] = nc.dram_tensor(f"fpad{m}", [NCH * B * 128, D], f32, kind="ExternalInput").ap()
        ins[f"ohc{m}"] = nc.dram_tensor(f"ohc{m}", [NCH, B * 128, 128], f32, kind="ExternalInput").ap()
        ins[f"ohg{m}"] = nc.dram_tensor(f"ohg{m}", [NCH, B * 128, 128], f32, kind="ExternalInput").ap()
        ins[f"acs{m}"] = nc.dram_tensor(f"acs{m}", [128, NCH], f32, kind="ExternalInput").ap()
        ins[f"ags{m}"] = nc.dram_tensor(f"ags{m}", [128, NCC], f32, kind="ExternalInput").ap()
        ins[f"cmem{m}"] = nc.dram_tensor(f"cmem{m}", [SEGS, D], f32, kind="ExternalInput").ap()
        ins[f"gmem{m}"] = nc.dram_tensor(f"gmem{m}", [CPC, D], f32, kind="ExternalInput").ap()
    out = nc.dram_tensor("out", [2 * (CPC + SEGS), D], f32, kind="ExternalOutput").ap()

    with tile.TileContext(nc) as tc:
        with tc.tile_pool(name="const", bufs=1) as constp, \
             tc.tile_pool(name="io", bufs=3) as iop, \
             tc.tile_pool(name="ohp", bufs=4) as ohp, \
             tc.tile_pool(name="psc", bufs=2, space="PSUM") as psc, \
             tc.tile_pool(name="psg", bufs=1, space="PSUM") as psg:

            for m in range(2):
                acs_t = constp.tile([128, NCH], f32, name=f"acs_t{m}")
                nc.sync.dma_start(out=acs_t[:], in_=ins[f"acs{m}"][:, :])
                ags_t = constp.tile([128, NCC], f32, name=f"ags_t{m}")
                nc.sync.dma_start(out=ags_t[:], in_=ins[f"ags{m}"][:, :])
                gbase = (CPC + SEGS) * m          # per-core out row offsets
                cbase = gbase + CPC

                for jc in range(NCC):
                    gpsum = psg.tile([128, D], f32, tag="gp", name="gpsum")
                    for si in range(6):
                        j = jc * 6 + si
                        cph = [psc.tile([128, 1024], f32, tag="cp", name=f"cph{h}")
                               for h in range(2)]
                        for b in range(B):
                            frow = iop.tile([128, D], f32, tag="frow", name="frow")
                            nc.sync.dma_start(
                                out=frow[:],
                                in_=ins[f"fpad{m}"][(j * B + b) * 128:(j * B + b + 1) * 128, :])
                            ohc_t = ohp.tile([128, 128], f32, tag="oh", name="ohc_t")
                            nc.sync.dma_start(out=ohc_t[:], in_=ins[f"ohc{m}"][j, b * 128:(b + 1) * 128, :])
                            ohg_t = ohp.tile([128, 128], f32, tag="oh", name="ohg_t")
                            nc.sync.dma_start(out=ohg_t[:], in_=ins[f"ohg{m}"][j, b * 128:(b + 1) * 128, :])
                            for t in range(4):
                                sl = slice(t * 512, (t + 1) * 512)
                                nc.tensor.matmul(
                                    cph[t // 2][:, (t % 2) * 512:(t % 2) * 512 + 512],
                                    ohc_t[:], frow[:, sl],
                                    start=(b == 0), stop=(b == B - 1))
                                nc.tensor.matmul(
                                    gpsum[:, sl],
                                    ohg_t[:], frow[:, sl],
                                    start=(si == 0 and b == 0), stop=(si == 5 and b == B - 1))
                        cmem_sb = iop.tile([128, D], f32, tag="cmem", name="cmem_sb")
                        nc.sync.dma_start(out=cmem_sb[:], in_=ins[f"cmem{m}"][j * 128:(j + 1) * 128, :])
                        outc_sb = iop.tile([128, D], f32, tag="outc", name="outc_sb")
                        for h in range(2):
                            hsl = slice(h * 1024, (h + 1) * 1024)
                            nc.vector.scalar_tensor_tensor(
                                out=outc_sb[:, hsl], in0=cmem_sb[:, hsl],
                                scalar=acs_t[:, j:j + 1], in1=cph[h][:],
                                op0=mybir.AluOpType.mult, op1=mybir.AluOpType.add)
                        nc.sync.dma_start(out=out[cbase + j * 128:cbase + (j + 1) * 128, :], in_=outc_sb[:])

                    gmem_sb = iop.tile([128, D], f32, tag="gmem", bufs=2, name="gmem_sb")
                    nc.sync.dma_start(out=gmem_sb[:], in_=ins[f"gmem{m}"][jc * 128:(jc + 1) * 128, :])
                    outg_sb = iop.tile([128, D], f32, tag="outg", bufs=2, name="outg_sb")
                    for h in range(2):
                        hsl = slice(h * 1024, (h + 1) * 1024)
                        nc.vector.scalar_tensor_tensor(
                            out=outg_sb[:, hsl], in0=gmem_sb[:, hsl],
                            scalar=ags_t[:, jc:jc + 1], in1=gpsum[:, hsl],
                            op0=mybir.AluOpType.mult, op1=mybir.AluOpType.add)
                    nc.sync.dma_start(out=out[gbase + jc * 128:gbase + (jc + 1) * 128, :], in_=outg_sb[:])

    nc.compile()
    return nc


def kernel(**inputs):
    from concourse.bass_utils import run_bass_kernel_spmd

    a = {k: np.ascontiguousarray(np.asarray(v)) for k, v in inputs.items()}
    modalities = [
        (a["rgb_feats"], np.asarray(a["rgb_labels"], np.int64), np.asarray(a["rgb_cams"], np.int64),
         a["vis_cam_valid"], a["vis_memory"], a["vis_cam_memory"]),
        (a["ir_feats"], np.asarray(a["ir_labels"], np.int64), np.asarray(a["ir_cams"], np.int64),
         a["ir_cam_valid"], a["ir_memory"], a["ir_cam_memory"]),
    ]

    # row capacity per 128-segment chunk: global max so one SPMD program serves all cores
    B = 1
    for feats, labels, cams, valid, gmem, cmem in modalities:
        for core in range(M):
            c0 = core * CPC
            msk = (labels >= c0) & (labels < c0 + CPC)
            seg = (labels[msk] - c0) * K + cams[msk]
            mx = np.bincount(seg // 128, minlength=NCH).max()
            B = max(B, int(np.ceil(mx / 128)))

    if B not in _BUILD_CACHE:
        _BUILD_CACHE[B] = _build_program(B)
    nc = _BUILD_CACHE[B]

    in_maps = []
    for core in range(M):
        im = {}
        for m, (feats, labels, cams, valid, gmem, cmem) in enumerate(modalities):
            d = _prep_core_modality(core, feats, labels, cams, valid, B)
            im[f"fpad{m}"] = d["fpad"]
            im[f"ohc{m}"] = d["ohc"]
            im[f"ohg{m}"] = d["ohg"]
            im[f"acs{m}"] = d["acs"]
            im[f"ags{m}"] = d["ags"]
            im[f"cmem{m}"] = np.ascontiguousarray(cmem.reshape(C * K, D)[core * SEGS:(core + 1) * SEGS])
            im[f"gmem{m}"] = np.ascontiguousarray(gmem[core * CPC:(core + 1) * CPC])
        in_maps.append(im)

    res = run_bass_kernel_spmd(nc, in_maps, core_ids=list(range(M)))

    CK = C * K
    full = np.empty((2 * C * (1 + K), D), F32)
    for core, r in enumerate(res.results):
        o = r["out"]
        full[core * CPC:(core + 1) * CPC] = o[:CPC]
        full[C + core * SEGS:C + (core + 1) * SEGS] = o[CPC:CPC + SEGS]
        full[C + CK + core * CPC:C + CK + (core + 1) * CPC] = o[CPC + SEGS:2 * CPC + SEGS]
        full[2 * C + CK + core * SEGS:2 * C + CK + (core + 1) * SEGS] = o[2 * CPC + SEGS:]
    return full


# revision 4
# speedup vs baseline: 1.6919x; 1.6919x over previous
"""Trainium2 Bass kernel for the CMA momentum-memory update (nn_CMA_52956946760162).

Strategy (class-sharded, present-only compact packing):
- Shard the C=4096 classes across 8 cores (512 classes/core), no collectives.
- Host packs, per (core, modality), the *present* (label,cam) segments and
  present labels into chunks of <=128 one-hot columns / <=128*B feature rows
  (whole classes per chunk). The one-hot entries are pre-scaled with the
  momentum/count coefficients (b_c = sigma_or_1/cnt, b_g = sigma/cnt), and a
  segment column and its class column share the same matmul, so one tensor-
  engine pass produces both per-(label,cam) and per-label scaled sums in PSUM.
- Host gathers the corresponding memory-bank rows densely (mem_in), so every
  device DMA is a dense [128 x 2048] f32 block. The device computes
  out = a * mem + psum in a single fused DVE op per chunk and streams it out.
- Rows absent from the batch leave memory unchanged; the host passes them
  through directly from the input banks during output assembly and scatters
  the device-computed rows over them.
"""

import numpy as np

C, K, D, N = 4096, 6, 2048, 16384
SIGMA = 0.2
M = 8                 # cores
CPC = C // M          # classes per core = 512
CK = C * K
F32 = np.float32

_BUILD_CACHE = {}


def _pack_core_modality(core, feats, labels, cams, valid, B, nch):
    """Pack one (core, modality) into chunk tensors.

    Returns fpad [nch*B*128, D], oh [nch, B*128, 128], avec [128, nch],
    mem_idx [nch, 128] (merged row id: class c -> c, seg s -> CPC + s, pad -> -1).
    """
    c0 = core * CPC
    mask = (labels >= c0) & (labels < c0 + CPC)
    rows_all = np.nonzero(mask)[0]
    lab = labels[rows_all] - c0
    seg = lab * K + cams[rows_all]
    order = np.argsort(seg, kind="stable")
    rows_all, lab, seg = rows_all[order], lab[order], seg[order]

    ccnt = np.bincount(seg, minlength=CPC * K).astype(F32)
    gcnt = np.bincount(lab, minlength=CPC).astype(F32)
    v = np.asarray(valid[c0:c0 + CPC]).reshape(CPC * K)
    a_c = np.where(v, 1.0 - SIGMA, 0.0).astype(F32)
    b_c = (np.where(v, SIGMA, 1.0) / np.maximum(ccnt, 1.0)).astype(F32)
    b_g = (SIGMA / np.maximum(gcnt, 1.0)).astype(F32)

    cpres = ccnt > 0
    class_start = np.searchsorted(lab, np.arange(CPC + 1))
    nseg_per_class = cpres.reshape(CPC, K).sum(axis=1)

    # greedy pack whole classes into chunks
    chunk_id = np.empty(len(rows_all), np.int64)    # per row
    slot = np.empty(len(rows_all), np.int64)        # row slot within chunk
    segcol_of = np.empty(CPC * K, np.int64)         # column of each present seg
    ccol_of = np.empty(CPC, np.int64)               # column of each present class
    chunk_of_class = np.empty(CPC, np.int64)
    mem_idx = np.full((nch, 128), -1, np.int64)
    avec = np.zeros((128, nch), F32)

    j, cols, rws = 0, 0, 0
    for c in np.nonzero(gcnt > 0)[0]:
        ns = int(nseg_per_class[c])
        r0, r1 = int(class_start[c]), int(class_start[c + 1])
        nr = r1 - r0
        if cols and (cols + ns + 1 > 128 or rws + nr > B * 128):
            j += 1
            cols, rws = 0, 0
        segs_c = np.nonzero(cpres[c * K:(c + 1) * K])[0] + c * K
        segcol_of[segs_c] = cols + np.arange(ns)
        ccol = cols + ns
        ccol_of[c] = ccol
        chunk_of_class[c] = j
        chunk_id[r0:r1] = j
        slot[r0:r1] = rws + np.arange(nr)
        mem_idx[j, cols:cols + ns] = CPC + segs_c
        mem_idx[j, ccol] = c
        avec[cols:cols + ns, j] = a_c[segs_c]
        avec[ccol, j] = 1.0 - SIGMA
        cols += ns + 1
        rws += nr
    assert (j + 1 if len(rows_all) else 0) <= nch

    fpad = np.zeros((nch, B * 128, D), F32)
    oh = np.zeros((nch, B * 128, 128), F32)
    fpad[chunk_id, slot] = feats[rows_all]
    oh[chunk_id, slot, segcol_of[seg]] = b_c[seg]
    oh[chunk_id, slot, ccol_of[lab]] = b_g[lab]
    return dict(fpad=fpad.reshape(nch * B * 128, D), oh=oh, avec=avec, mem_idx=mem_idx)


def _chunk_stats(labels, cams):
    """Per core: max rows per class (for B) and chunk count (for nch)."""
    out = []
    for core in range(M):
        c0 = core * CPC
        mask = (labels >= c0) & (labels < c0 + CPC)
        lab = labels[mask] - c0
        seg = lab * K + cams[mask]
        gcnt = np.bincount(lab, minlength=CPC)
        ccnt = np.bincount(seg, minlength=CPC * K)
        out.append((int(gcnt.max()), gcnt, (ccnt > 0).reshape(CPC, K).sum(axis=1)))
    return out


def _count_chunks(gcnt, nseg_per_class, B):
    j, cols, rws, any_rows = 0, 0, 0, False
    for c in np.nonzero(gcnt > 0)[0]:
        ns = int(nseg_per_class[c])
        nr = int(gcnt[c])
        if cols and (cols + ns + 1 > 128 or rws + nr > B * 128):
            j += 1
            cols, rws = 0, 0
        cols += ns + 1
        rws += nr
        any_rows = True
    return j + 1 if any_rows else 0


def _build_program(B, nch):
    """Build + compile the SPMD Bass program; 2*nch chunks (both modalities)."""
    import concourse.mybir as mybir
    import concourse.tile as tile
    from concourse import bacc

    f32 = mybir.dt.float32
    nc = bacc.Bacc("TRN2", target_bir_lowering=False, debug=False)

    NT = 2 * nch
    fpad = nc.dram_tensor("fpad", [NT * B * 128, D], f32, kind="ExternalInput").ap()
    oh = nc.dram_tensor("oh", [NT, B * 128, 128], f32, kind="ExternalInput").ap()
    memin = nc.dram_tensor("memin", [NT * 128, D], f32, kind="ExternalInput").ap()
    avec = nc.dram_tensor("avec", [128, NT], f32, kind="ExternalInput").ap()
    out = nc.dram_tensor("out", [NT * 128, D], f32, kind="ExternalOutput").ap()

    with tile.TileContext(nc) as tc:
        with tc.tile_pool(name="const", bufs=1) as constp, \
             tc.tile_pool(name="io", bufs=5) as iop, \
             tc.tile_pool(name="ohp", bufs=5) as ohp, \
             tc.tile_pool(name="ps", bufs=2, space="PSUM") as psp:

            avec_t = constp.tile([128, NT], f32, name="avec_t")
            nc.sync.dma_start(out=avec_t[:], in_=avec[:, :])

            for j in range(NT):
                psum = psp.tile([128, D], f32, tag="ps", name="psum")
                for b in range(B):
                    r0 = (j * B + b) * 128
                    frow = iop.tile([128, D], f32, tag="frow", name="frow")
                    nc.sync.dma_start(out=frow[:], in_=fpad[r0:r0 + 128, :])
                    oht = ohp.tile([128, 128], f32, tag="oh", name="oht")
                    nc.sync.dma_start(out=oht[:], in_=oh[j, b * 128:(b + 1) * 128, :])
                    for t in range(4):
                        sl = slice(t * 512, (t + 1) * 512)
                        nc.tensor.matmul(psum[:, sl], oht[:], frow[:, sl],
                                         start=(b == 0), stop=(b == B - 1))
                mem_sb = iop.tile([128, D], f32, tag="mem", name="mem_sb")
                nc.sync.dma_start(out=mem_sb[:], in_=memin[j * 128:(j + 1) * 128, :])
                out_sb = iop.tile([128, D], f32, tag="out", name="out_sb")
                nc.vector.scalar_tensor_tensor(
                    out=out_sb[:], in0=mem_sb[:], scalar=avec_t[:, j:j + 1],
                    in1=psum[:], op0=mybir.AluOpType.mult, op1=mybir.AluOpType.add)
                nc.sync.dma_start(out=out[j * 128:(j + 1) * 128, :], in_=out_sb[:])

    nc.compile()
    return nc


def prepare(inputs):
    """Build (or reuse) the program and the per-core input maps + scatter metadata."""
    a = {k: np.ascontiguousarray(np.asarray(v)) for k, v in inputs.items()}
    mods = [
        (a["rgb_feats"], a["rgb_labels"].astype(np.int64), a["rgb_cams"].astype(np.int64),
         a["vis_cam_valid"], a["vis_memory"], a["vis_cam_memory"].reshape(CK, D)),
        (a["ir_feats"], a["ir_labels"].astype(np.int64), a["ir_cams"].astype(np.int64),
         a["ir_cam_valid"], a["ir_memory"], a["ir_cam_memory"].reshape(CK, D)),
    ]

    # global B and chunk count (uniform across cores -> one SPMD program)
    B, nch = 1, 1
    stats = []
    for feats, labels, cams, valid, gmem, cmem in mods:
        st = _chunk_stats(labels, cams)
        stats.append(st)
        for mx, _, _ in st:
            B = max(B, int(np.ceil(mx / 128)))
    for st in stats:
        for _, gcnt, nspc in st:
            nch = max(nch, _count_chunks(gcnt, nspc, B))

    key = (B, nch)
    if key not in _BUILD_CACHE:
        _BUILD_CACHE[key] = _build_program(B, nch)
    nc = _BUILD_CACHE[key]

    in_maps, metas = [], []
    for core in range(M):
        c0 = core * CPC
        packs = []
        for m, (feats, labels, cams, valid, gmem, cmem) in enumerate(mods):
            packs.append(_pack_core_modality(core, feats, labels, cams, valid, B, nch))
        im = {
            "fpad": np.concatenate([p["fpad"] for p in packs], axis=0),
            "oh": np.concatenate([p["oh"] for p in packs], axis=0),
            "avec": np.concatenate([p["avec"] for p in packs], axis=1),
        }
        memin = np.zeros((2 * nch * 128, D), F32)
        meta = []
        for m, p in enumerate(packs):
            gmem, cmem = mods[m][4], mods[m][5]
            idx = p["mem_idx"].reshape(nch * 128)
            used = np.nonzero(idx >= 0)[0]
            gidx = idx[used]
            isg = gidx < CPC
            src = np.where(isg, c0 + gidx, core * CPC * K + (gidx - CPC))
            block = memin[m * nch * 128:(m + 1) * nch * 128]
            block[used[isg]] = gmem[src[isg]]
            block[used[~isg]] = cmem[src[~isg]]
            obase = (C + CK) * m
            tgt = np.where(isg, obase + c0 + gidx,
                           obase + C + core * CPC * K + (gidx - CPC))
            meta.append((used + m * nch * 128, tgt))
        im["memin"] = memin
        in_maps.append(im)
        metas.append(meta)
    return nc, in_maps, metas, a, mods


def assemble(a, mods, metas, results):
    full = np.concatenate([a["vis_memory"], mods[0][5], a["ir_memory"], mods[1][5]],
                          axis=0).astype(F32, copy=True)
    for core in range(M):
        o = results[core]["out"]
        for used, tgt in metas[core]:
            full[tgt] = o[used]
    return full


def kernel(**inputs):
    from concourse.bass_utils import run_bass_kernel_spmd

    nc, in_maps, metas, a, mods = prepare(inputs)
    res = run_bass_kernel_spmd(nc, in_maps, core_ids=list(range(M)))
    return assemble(a, mods, metas, res.results)
